# revision 15
# baseline (speedup 1.0000x reference)
"""Trainium2 Bass kernel for a 4-layer IndRNN (B=32, T=2048, I=256, H=512).

Math: per layer, xp = x @ W.T + b, then the per-channel recurrence
    h_t = relu(xp_t + w * h_{t-1}),  w = whs[l] in [0, 1)

Since w >= 0, the nonlinear scan decomposes into two DVE tensor_tensor_scan
passes plus one cheap elementwise subtract:
    dloc_t = w * dloc_{t-1} + xp_t          (linear scan; dloc_{-1} = 0)
    u_t    = min(w * u_{t-1}, dloc_t)       (min-scan;    u_{-1} = 0)
    h_t    = dloc_t - u_t                   (>= 0 by construction: no relu!)
Proof: with s_t = xp_t + w*relu(s_{t-1}) (h=relu(s)), put s_t = dloc_t + r_t;
then q_t := -r_t satisfies q_t = w*min(dloc_{t-1}, q_{t-1}) and
h = relu(dloc - q) = dloc - min(q, dloc) =: dloc - u, where u_t =
min(q_t, dloc_t) = min(w*u_{t-1}, dloc_t). Verified exactly vs the
sequential reference in fp64.

vs the previous revision this removes the scalar_tensor_tensor pass
(2.27us/tile DVE) and the ACT relu entirely; the subtract runs in DVE 2x
mode (~1.2us/tile, all-fp16 operands). GpSimd/Pool is left idle on
purpose: measured on HW, Pool tensor ops contend on the DVE/GPSIMD shared
SBUF ports and slow DVE scans ~2x, a strict loss (and codegen rejects
tensor_tensor_scan/scalar_tensor_tensor on Pool anyway).

Scheduling: batches are processed in interleaved PAIRS so that every
engine always has an independent tile stream: while PE matmuls layer l of
batch b1, DVE scans layer l of batch b0, and layer boundaries never stall
either engine. DVE measures 99% busy; the two tensor_tensor_scan passes
(4.41us per [128,2048] tile: a fixed 2 cycles/element, no DVE perf modes
exist for scans) are the roofline for this decomposition.

Sharding: data-parallel over batch, 4 batches per core, weights replicated.
Layout on device: [H(partitions), T(free)] per batch; the host pre-transposes
the layer-0 input to [I, T] and post-transposes the output from [H, T], so the
device never pays for transposes.
"""

import numpy as np
from contextlib import ExitStack

import concourse.bass as bass
import concourse.tile as tile
from concourse import mybir
from concourse.bass_utils import run_bass_kernel_spmd

dt = mybir.dt
Alu = mybir.AluOpType

B, T, I, H, L = 32, 2048, 256, 512, 4
NCORES = 8
BLOC = B // NCORES
P = 128
TCH = 512  # matmul PSUM chunk (one bank of fp32)


def build(bloc=BLOC, t=T, include_bias=False, trace_sim=False):
    """Build the per-core Bass program (SPMD; identical on all cores)."""
    assert t % TCH == 0
    nch = t // TCH
    ki, kh, m4 = I // P, H // P, H // P

    nc = bass.Bass("TRN2", target_bir_lowering=False, debug=False,
                   num_devices=NCORES)
    xT_d = nc.dram_tensor("xT", [bloc, I, t], dt.float16, kind="ExternalInput").ap()
    w0t_d = nc.dram_tensor("w0t", [I, H], dt.float16, kind="ExternalInput").ap()
    wst_d = nc.dram_tensor("wst", [L - 1, H, H], dt.float16, kind="ExternalInput").ap()
    bias_d = nc.dram_tensor("bias", [L, 1, H], dt.float16, kind="ExternalInput").ap()
    wbc_d = nc.dram_tensor("wbc", [L, H, 1], dt.float32, kind="ExternalInput").ap()
    out_d = nc.dram_tensor("out", [bloc, H, t], dt.float16, kind="ExternalOutput").ap()

    with tile.TileContext(nc, trace_sim=trace_sim) as tc, ExitStack() as ctx:
        wpool = ctx.enter_context(tc.tile_pool(name="weights", bufs=1))
        xpool = ctx.enter_context(tc.tile_pool(name="xin", bufs=2 * BLOC))
        hpool = ctx.enter_context(tc.tile_pool(name="h", bufs=12))
        dpool = ctx.enter_context(tc.tile_pool(name="dloc", bufs=3))
        upool = ctx.enter_context(tc.tile_pool(name="u", bufs=3))
        opool = ctx.enter_context(tc.tile_pool(name="hout", bufs=4))
        psum = ctx.enter_context(tc.tile_pool(name="psum", bufs=2, space="PSUM"))

        # --- persistent weights (SWDGE DMAs from the Pool sequencer: keeps
        # all 8 HWDGE queues free so each output store carries only its
        # single DVE data wait). DMAs are issued in FIRST-USE order (layer-0
        # weights + first pair's inputs up front) so the pipeline starts
        # ~30us earlier than a bulk-load order would.
        # lhsT tiles [K=128, M<=512]; lhsT slice [:, m*128:(m+1)*128] per matmul
        wt = [[] for _ in range(L)]   # wt[l][k] -> [128, H] fp16
        wbc = [[] for _ in range(L)]  # wbc[l][m] -> [128, 1] fp32
        xtiles = {}                   # b -> [ki tiles]

        def load_layer(l):
            for m in range(m4):
                w = wpool.tile([P, 1], dt.float32, tag=f"wb{l}{m}")
                nc.gpsimd.dma_start(
                    out=w[:], in_=wbc_d[l, m * P:(m + 1) * P, :])
                wbc[l].append(w)
            for k in range(ki if l == 0 else kh):
                w = wpool.tile([P, H], dt.float16, tag=f"w{l}{k}")
                src = w0t_d[k * P:(k + 1) * P, :] if l == 0 else \
                    wst_d[l - 1, k * P:(k + 1) * P, :]
                nc.gpsimd.dma_start(out=w[:], in_=src)
                wt[l].append(w)

        def load_x(b):
            tls = []
            for k in range(ki):
                xt = xpool.tile([P, t], dt.float16, tag="xin")
                nc.gpsimd.dma_start(out=xt[:], in_=xT_d[b, k * P:(k + 1) * P, :])
                tls.append(xt)
            xtiles[b] = tls

        load_layer(0)
        load_x(0)
        load_x(1)
        for l in range(1, L):
            load_layer(l)
        for b in range(2, bloc):
            load_x(b)
        if include_bias:
            bias = []
            for l in range(L):
                bt = wpool.tile([1, H], dt.float16, tag=f"b{l}")
                nc.gpsimd.dma_start(out=bt[:], in_=bias_d[l, :, :])
                bias.append(bt)
            ones = wpool.tile([1, TCH], dt.float16, tag="ones")
            nc.gpsimd.memset(ones[:], 1.0)
        # Non-PE instructions can carry only ONE sync-wait through walrus
        # codegen. Same-engine waits merge into one semaphore, so each engine
        # touches every cross-engine dependency in a cheap "claimer" op
        # first, leaving every real op a single wait. Per layer (emitted
        # lazily at the layer's first tile so the PE doesn't block on later
        # layers' weight DMAs at startup):
        #  - DVE touches the layer's wbc tiles (scan operands),
        #  - PE runs junk ldweights per weight tile (no PSUM write, no WAW).
        scratch = wpool.tile([P, L * m4 + 1], dt.float32, tag="scratch")

        def preamble_layer(l):
            for m in range(m4):
                col = slice(l * m4 + m, l * m4 + m + 1)
                nc.vector.tensor_copy(scratch[:, col], wbc[l][m][:, 0:1])
            for k in range(len(wt[l])):
                nc.tensor.ldweights(weights=wt[l][k][:, 0:P])

        if include_bias:
            for l in range(L):
                nc.tensor.ldweights(weights=bias[l][:, 0:P])
            nc.tensor.ldweights(weights=ones[:, 0:P])

        # --- main loop: batch-PAIR interleaved ---
        houts = {}
        prev = {}      # b -> list of input tiles for the next layer
        building = {}  # b -> h tiles of the layer currently being produced
        xp_count = 0
        xp_readers = {}  # psum slot -> last scan1 instruction that read it
        order = []
        for bp in range(bloc // 2):
            for l in range(L):
                for b in (2 * bp, 2 * bp + 1):
                    for m in range(m4):
                        order.append((b, l, m))
        preambled = set()
        for (b, l, m) in order:
            if l not in preambled:
                preambled.add(l)
                preamble_layer(l)
            pv = prev[b] if l > 0 else xtiles[b]
            kprev = len(pv)
            xp = psum.tile([P, t], dt.float32, tag="xp")
            # PE claimer ldweights (junk loads, no PSUM write): one absorbs
            # the DVE scan tick guarding the recycled PSUM slot (forced
            # dep), the m==0 extras absorb the rhs producer tick (input DMA
            # for layer 0, DVE subtract for later layers).
            old_rd = xp_readers.get(xp_count % 2)
            xp_count += 1
            claimers = []
            if old_rd is not None:
                ldw = nc.tensor.ldweights(weights=wt[l][0][:, 0:P])
                bass._add_dep_helper(
                    ldw.ins, old_rd.ins, sync=True,
                    reason="PE DVE-clock claimer for PSUM slot WAR")
                claimers.append(ldw)
            if m == 0:
                for kc in range(kprev if l == 0 else 1):
                    claimers.append(nc.tensor.ldweights(
                        weights=pv[kprev - 1 - kc][:, 0:P]))
            last_mm = None
            for n in range(nch):
                ns = slice(n * TCH, (n + 1) * TCH)
                for k in range(kprev):
                    last_mm = nc.tensor.matmul(
                        xp[:, ns], lhsT=wt[l][k][:, m * P:(m + 1) * P],
                        rhs=pv[k][:, ns],
                        start=(k == 0),
                        stop=(k == kprev - 1 and not include_bias))
                    for cl in claimers:  # pin claimers before 1st MM
                        bass._add_dep_helper(
                            last_mm.ins, cl.ins, sync=False,
                            reason="order claimer before real MMs")
                    claimers = []
                if include_bias:
                    last_mm = nc.tensor.matmul(
                        xp[:, ns], lhsT=bias[l][:, m * P:(m + 1) * P],
                        rhs=ones[:, :], start=False, stop=True)
            # scan1: dloc_t = w*dloc_{t-1} + xp_t. Its only cross-engine
            # dep is the matmul (wbc absorbed in preamble; the recycled
            # dloc slot's last reader is the DVE subtract -> same engine).
            dloc = dpool.tile([P, t], dt.float16, tag="dloc")
            wb_full = wbc[l][m][:, 0:1].broadcast_to((P, t))
            scan1 = nc.vector.tensor_tensor_scan(
                out=dloc[:], data0=wb_full, data1=xp[:],
                initial=0.0, op0=Alu.mult, op1=Alu.add)
            xp_readers[(xp_count - 1) % 2] = scan1
            # scan2: u_t = min(w*u_{t-1}, dloc_t). All deps same-engine.
            u = upool.tile([P, t], dt.float16, tag="u")
            nc.vector.tensor_tensor_scan(
                out=u[:], data0=wb_full, data1=dloc[:],
                initial=0.0, op0=Alu.mult, op1=Alu.min)
            # h = dloc - u (DVE 2x mode, all fp16). The recycled h slot's
            # last reader is a PE matmul: claim it with a [P,1] memset
            # pinned on this tile's last matmul (PE order covers any
            # earlier reader of the slot).
            if l < L - 1:
                h = hpool.tile([P, t], dt.float16, tag="h")
                cl = nc.vector.memset(h[:, 0:1], 0.0)
                bass._add_dep_helper(
                    cl.ins, last_mm.ins, sync=True,
                    reason="DVE PE-clock claimer for h slot WAR")
                nc.vector.tensor_tensor(
                    out=h[:], in0=dloc[:], in1=u[:], op=Alu.subtract)
                if m == 0:
                    building[b] = []
                building[b].append(h)
                if m == m4 - 1:
                    prev[b] = building[b]
            else:
                # Final layer: outputs for the two batches of a pair share
                # one [P, 2t] tile and go out in ONE DMA (8 stores total =
                # one per HWDGE queue).
                if b % 2 == 0:
                    h2 = opool.tile([P, 2 * t], dt.float16, tag="hout")
                    houts[m] = h2
                    cl = nc.vector.memset(h2[:, 0:1], 0.0)
                    bass._add_dep_helper(
                        cl.ins, last_mm.ins, sync=True,
                        reason="DVE PE-clock claimer for hout slot WAR")
                h2 = houts[m]
                nc.vector.tensor_tensor(
                    out=h2[:, (b % 2) * t:(b % 2 + 1) * t],
                    in0=dloc[:], in1=u[:], op=Alu.subtract)
                if b % 2 == 1:
                    dst = out_d[b - 1:b + 1, m * P:(m + 1) * P, :]
                    nc.sync.dma_start(
                        out=dst.rearrange("b p t -> p b t"),
                        in_=h2[:].rearrange("p (b t) -> p b t", b=2))
        # Tail pre-drain: the auto kernel-tail drain on SP must observe
        # every DMA queue and engine tick; feed SP one dependency per
        # pre-drain (same-proc waits merge) so the auto drain ends at zero.
        tail_deps = [i for i in nc.inst_map.values()
                     if type(i).__name__ == "InstDMACopy"]
        _ctl = {"InstUnconditionalBranch", "InstRegisterMove", "InstDrain",
                "InstEventSemaphore", "InstCall", "InstCompareBranch"}
        snap = [i for i in nc.inst_map.values()
                if type(i).__name__ not in _ctl]
        for eng in ("DVE", "Activation"):
            last_e = [i for i in snap
                      if str(getattr(i, "engine", "")).endswith(eng)]
            if last_e:
                tail_deps.append(last_e[-1])
        tail_deps += [last_mm.ins, scan1.ins]
        for depi in tail_deps:
            dr = nc.sync.drain(fusable=False)
            bass._add_dep_helper(dr.ins, depi, sync=True,
                                 reason="tail pre-drain absorber")
    _assert_wait_budget(nc)
    return nc


# Instruction families exempt from the 1-sync-wait TPB events header (DMA
# descriptors and drains use the queue sync machinery). Everything that runs
# on a TPB engine sequencer (PE/DVE/ACT/Pool alike) is capacity-1.
_MULTI_WAIT_OK = {"InstDrain",
                  "InstEventSemaphore", "InstUnconditionalBranch",
                  "InstRegisterMove", "InstISA", "InstTensorLoad",
                  "InstTensorSave"}


def _assert_wait_budget(nc):
    bad = []
    for name, inst in nc.inst_map.items():
        ty = type(inst).__name__
        if ty in _MULTI_WAIT_OK:
            continue
        w = inst.sync_info.on_wait if inst.sync_info else []
        if len(w) > 1:
            bad.append((name, ty,
                        [f"{x.ant_name}>={x.wait_value}" for x in w]))
    if bad:
        raise RuntimeError(
            f"{len(bad)} instructions exceed the 1-sync-wait TPB limit, "
            f"first few: {bad[:5]}")


def _prep_core_inputs(Input, W0, Ws, bs, whs, core):
    """Host-side staging for one core: shard batch, transpose layer-0 input,
    pre-transpose weights into lhsT layout."""
    bsl = slice(core * BLOC, (core + 1) * BLOC)
    return {
        "xT": np.ascontiguousarray(
            Input[bsl].transpose(0, 2, 1)).astype(np.float16),
        "w0t": np.ascontiguousarray(W0.T).astype(np.float16),
        "wst": np.ascontiguousarray(Ws.transpose(0, 2, 1)).astype(np.float16),
        "bias": np.ascontiguousarray(bs[:, None, :]).astype(np.float16),
        "wbc": np.ascontiguousarray(whs.astype(np.float32)[:, :, None]),
    }


def kernel(Input, W0, Ws, bs, whs):
    include_bias = bool(np.any(bs != 0))
    nc = build(include_bias=include_bias)
    in_maps = [_prep_core_inputs(Input, W0, Ws, bs, whs, r)
               for r in range(NCORES)]
    res = run_bass_kernel_spmd(nc, in_maps, core_ids=list(range(NCORES)))
    parts = [res.results[r]["out"] for r in range(NCORES)]  # [BLOC, H, T] each
    full = np.concatenate(parts, axis=0)  # [B, H, T]
    return np.ascontiguousarray(full.transpose(0, 2, 1)).astype(np.float32)


# revision 16
# speedup vs baseline: 1.0078x; 1.0078x over previous
"""Trainium2 Bass kernel for a 4-layer IndRNN (B=32, T=2048, I=256, H=512).

Math: per layer, xp = x @ W.T + b, then the per-channel recurrence
    h_t = relu(xp_t + w * h_{t-1}),  w = whs[l] in [0, 1)

Since w >= 0, the nonlinear scan decomposes into two DVE tensor_tensor_scan
passes plus one cheap elementwise subtract:
    dloc_t = w * dloc_{t-1} + xp_t          (linear scan; dloc_{-1} = 0)
    u_t    = min(w * u_{t-1}, dloc_t)       (min-scan;    u_{-1} = 0)
    h_t    = dloc_t - u_t                   (>= 0 by construction: no relu!)
Proof: with s_t = xp_t + w*relu(s_{t-1}) (h=relu(s)), put s_t = dloc_t + r_t;
then q_t := -r_t satisfies q_t = w*min(dloc_{t-1}, q_{t-1}) and
h = relu(dloc - q) = dloc - min(q, dloc) =: dloc - u, where u_t =
min(q_t, dloc_t) = min(w*u_{t-1}, dloc_t). Verified exactly vs the
sequential reference in fp64.

vs the previous revision this removes the scalar_tensor_tensor pass
(2.27us/tile DVE) and the ACT relu entirely; the subtract runs in DVE 2x
mode (~1.2us/tile, all-fp16 operands). GpSimd/Pool is left idle on
purpose: measured on HW, Pool tensor ops contend on the DVE/GPSIMD shared
SBUF ports and slow DVE scans ~2x, a strict loss (and codegen rejects
tensor_tensor_scan/scalar_tensor_tensor on Pool anyway).

Scheduling: batches are processed in interleaved PAIRS so that every
engine always has an independent tile stream: while PE matmuls layer l of
batch b1, DVE scans layer l of batch b0, and layer boundaries never stall
either engine. DVE measures 99% busy; the two tensor_tensor_scan passes
(4.41us per [128,2048] tile: a fixed 2 cycles/element, no DVE perf modes
exist for scans) are the roofline for this decomposition.

Sharding: data-parallel over batch, 4 batches per core, weights replicated.
Layout on device: [H(partitions), T(free)] per batch; the host pre-transposes
the layer-0 input to [I, T] and post-transposes the output from [H, T], so the
device never pays for transposes.
"""

import numpy as np
from contextlib import ExitStack

import concourse.bass as bass
import concourse.tile as tile
from concourse import mybir
from concourse.bass_utils import run_bass_kernel_spmd

dt = mybir.dt
Alu = mybir.AluOpType

B, T, I, H, L = 32, 2048, 256, 512, 4
NCORES = 8
BLOC = B // NCORES
P = 128
TCH = 512  # matmul PSUM chunk (one bank of fp32)


def build(bloc=BLOC, t=T, include_bias=False, trace_sim=False):
    """Build the per-core Bass program (SPMD; identical on all cores)."""
    assert t % TCH == 0
    nch = t // TCH
    ki, kh, m4 = I // P, H // P, H // P

    nc = bass.Bass("TRN2", target_bir_lowering=False, debug=False,
                   num_devices=NCORES)
    xT_d = nc.dram_tensor("xT", [bloc, I, t], dt.float16, kind="ExternalInput").ap()
    w0t_d = nc.dram_tensor("w0t", [I, H], dt.float16, kind="ExternalInput").ap()
    wst_d = nc.dram_tensor("wst", [L - 1, H, H], dt.float16, kind="ExternalInput").ap()
    bias_d = nc.dram_tensor("bias", [L, 1, H], dt.float16, kind="ExternalInput").ap()
    wbc_d = nc.dram_tensor("wbc", [L, H, 1], dt.float32, kind="ExternalInput").ap()
    out_d = nc.dram_tensor("out", [bloc, H, t], dt.float16, kind="ExternalOutput").ap()

    with tile.TileContext(nc, trace_sim=trace_sim) as tc, ExitStack() as ctx:
        wpool = ctx.enter_context(tc.tile_pool(name="weights", bufs=1))
        xpool = ctx.enter_context(tc.tile_pool(name="xin", bufs=2 * BLOC))
        hpool = ctx.enter_context(tc.tile_pool(name="h", bufs=12))
        dpool = ctx.enter_context(tc.tile_pool(name="dloc", bufs=3))
        upool = ctx.enter_context(tc.tile_pool(name="u", bufs=3))
        opool = ctx.enter_context(tc.tile_pool(name="hout", bufs=4))
        psum = ctx.enter_context(tc.tile_pool(name="psum", bufs=2, space="PSUM"))

        # --- persistent weights (SWDGE DMAs from the Pool sequencer: keeps
        # all 8 HWDGE queues free so each output store carries only its
        # single DVE data wait). DMAs are issued in FIRST-USE order (layer-0
        # weights + first pair's inputs up front) so the pipeline starts
        # ~30us earlier than a bulk-load order would.
        # lhsT tiles [K=128, M<=512]; lhsT slice [:, m*128:(m+1)*128] per matmul
        wt = [[] for _ in range(L)]   # wt[l][k] -> [128, H] fp16
        wbc = [[] for _ in range(L)]  # wbc[l][m] -> [128, 1] fp32
        xtiles = {}                   # b -> [ki tiles]

        def load_w(l):
            for k in range(ki if l == 0 else kh):
                w = wpool.tile([P, H], dt.float16, tag=f"w{l}{k}")
                src = w0t_d[k * P:(k + 1) * P, :] if l == 0 else \
                    wst_d[l - 1, k * P:(k + 1) * P, :]
                nc.gpsimd.dma_start(out=w[:], in_=src)
                wt[l].append(w)

        def load_wbc(l):
            for m in range(m4):
                w = wpool.tile([P, 1], dt.float32, tag=f"wb{l}{m}")
                nc.gpsimd.dma_start(
                    out=w[:], in_=wbc_d[l, m * P:(m + 1) * P, :])
                wbc[l].append(w)

        def load_x(b):
            tls = []
            for k in range(ki):
                xt = xpool.tile([P, t], dt.float16, tag="xin")
                nc.gpsimd.dma_start(out=xt[:], in_=xT_d[b, k * P:(k + 1) * P, :])
                tls.append(xt)
            xtiles[b] = tls

        # issue order = first-use order; the first tile's matmuls need
        # w(l0) + x(b0), so those go ahead of the (issue-costly) tiny wbc
        # transfers, which are only needed once the first scan starts.
        load_w(0)
        load_x(0)
        load_wbc(0)
        load_x(1)
        for l in range(1, L):
            load_w(l)
            load_wbc(l)
        for b in range(2, bloc):
            load_x(b)
        if include_bias:
            bias = []
            for l in range(L):
                bt = wpool.tile([1, H], dt.float16, tag=f"b{l}")
                nc.gpsimd.dma_start(out=bt[:], in_=bias_d[l, :, :])
                bias.append(bt)
            ones = wpool.tile([1, TCH], dt.float16, tag="ones")
            nc.gpsimd.memset(ones[:], 1.0)
        # Non-PE instructions can carry only ONE sync-wait through walrus
        # codegen. Same-engine waits merge into one semaphore, so each engine
        # touches every cross-engine dependency in a cheap "claimer" op
        # first, leaving every real op a single wait. Per layer (emitted
        # lazily at the layer's first tile so the PE doesn't block on later
        # layers' weight DMAs at startup):
        #  - DVE touches the layer's wbc tiles (scan operands),
        #  - PE runs junk ldweights per weight tile (no PSUM write, no WAW).
        scratch = wpool.tile([P, L * m4 + 1], dt.float32, tag="scratch")

        def preamble_layer(l):
            for m in range(m4):
                col = slice(l * m4 + m, l * m4 + m + 1)
                nc.vector.tensor_copy(scratch[:, col], wbc[l][m][:, 0:1])
            for k in range(len(wt[l])):
                nc.tensor.ldweights(weights=wt[l][k][:, 0:P])

        if include_bias:
            for l in range(L):
                nc.tensor.ldweights(weights=bias[l][:, 0:P])
            nc.tensor.ldweights(weights=ones[:, 0:P])

        # --- main loop: batch-PAIR interleaved ---
        houts = {}
        prev = {}      # b -> list of input tiles for the next layer
        building = {}  # b -> h tiles of the layer currently being produced
        xp_count = 0
        xp_readers = {}  # psum slot -> last scan1 instruction that read it
        order = []
        for bp in range(bloc // 2):
            for l in range(L):
                for b in (2 * bp, 2 * bp + 1):
                    for m in range(m4):
                        order.append((b, l, m))
        preambled = set()
        for (b, l, m) in order:
            if l not in preambled:
                preambled.add(l)
                preamble_layer(l)
            pv = prev[b] if l > 0 else xtiles[b]
            kprev = len(pv)
            xp = psum.tile([P, t], dt.float32, tag="xp")
            # PE claimer ldweights (junk loads, no PSUM write): one absorbs
            # the DVE scan tick guarding the recycled PSUM slot (forced
            # dep), the m==0 extras absorb the rhs producer tick (input DMA
            # for layer 0, DVE subtract for later layers).
            old_rd = xp_readers.get(xp_count % 2)
            xp_count += 1
            claimers = []
            if old_rd is not None:
                ldw = nc.tensor.ldweights(weights=wt[l][0][:, 0:P])
                bass._add_dep_helper(
                    ldw.ins, old_rd.ins, sync=True,
                    reason="PE DVE-clock claimer for PSUM slot WAR")
                claimers.append(ldw)
            if m == 0:
                for kc in range(kprev if l == 0 else 1):
                    claimers.append(nc.tensor.ldweights(
                        weights=pv[kprev - 1 - kc][:, 0:P]))
            last_mm = None
            for n in range(nch):
                ns = slice(n * TCH, (n + 1) * TCH)
                for k in range(kprev):
                    last_mm = nc.tensor.matmul(
                        xp[:, ns], lhsT=wt[l][k][:, m * P:(m + 1) * P],
                        rhs=pv[k][:, ns],
                        start=(k == 0),
                        stop=(k == kprev - 1 and not include_bias))
                    for cl in claimers:  # pin claimers before 1st MM
                        bass._add_dep_helper(
                            last_mm.ins, cl.ins, sync=False,
                            reason="order claimer before real MMs")
                    claimers = []
                if include_bias:
                    last_mm = nc.tensor.matmul(
                        xp[:, ns], lhsT=bias[l][:, m * P:(m + 1) * P],
                        rhs=ones[:, :], start=False, stop=True)
            # scan1: dloc_t = w*dloc_{t-1} + xp_t. Its only cross-engine
            # dep is the matmul (wbc absorbed in preamble; the recycled
            # dloc slot's last reader is the DVE subtract -> same engine).
            dloc = dpool.tile([P, t], dt.float16, tag="dloc")
            wb_full = wbc[l][m][:, 0:1].broadcast_to((P, t))
            scan1 = nc.vector.tensor_tensor_scan(
                out=dloc[:], data0=wb_full, data1=xp[:],
                initial=0.0, op0=Alu.mult, op1=Alu.add)
            xp_readers[(xp_count - 1) % 2] = scan1
            # scan2: u_t = min(w*u_{t-1}, dloc_t). All deps same-engine.
            u = upool.tile([P, t], dt.float16, tag="u")
            nc.vector.tensor_tensor_scan(
                out=u[:], data0=wb_full, data1=dloc[:],
                initial=0.0, op0=Alu.mult, op1=Alu.min)
            # h = dloc - u (DVE 2x mode, all fp16). The recycled h slot's
            # last reader is a PE matmul: claim it with a [P,1] memset
            # pinned on this tile's last matmul (PE order covers any
            # earlier reader of the slot).
            if l < L - 1:
                h = hpool.tile([P, t], dt.float16, tag="h")
                cl = nc.vector.memset(h[:, 0:1], 0.0)
                bass._add_dep_helper(
                    cl.ins, last_mm.ins, sync=True,
                    reason="DVE PE-clock claimer for h slot WAR")
                nc.vector.tensor_tensor(
                    out=h[:], in0=dloc[:], in1=u[:], op=Alu.subtract)
                if m == 0:
                    building[b] = []
                building[b].append(h)
                if m == m4 - 1:
                    prev[b] = building[b]
            else:
                # Final layer: outputs for the two batches of a pair share
                # one [P, 2t] tile and go out in ONE DMA (8 stores total =
                # one per HWDGE queue).
                if b % 2 == 0:
                    h2 = opool.tile([P, 2 * t], dt.float16, tag="hout")
                    houts[m] = h2
                    cl = nc.vector.memset(h2[:, 0:1], 0.0)
                    bass._add_dep_helper(
                        cl.ins, last_mm.ins, sync=True,
                        reason="DVE PE-clock claimer for hout slot WAR")
                h2 = houts[m]
                nc.vector.tensor_tensor(
                    out=h2[:, (b % 2) * t:(b % 2 + 1) * t],
                    in0=dloc[:], in1=u[:], op=Alu.subtract)
                if b % 2 == 1:
                    dst = out_d[b - 1:b + 1, m * P:(m + 1) * P, :]
                    nc.sync.dma_start(
                        out=dst.rearrange("b p t -> p b t"),
                        in_=h2[:].rearrange("p (b t) -> p b t", b=2))
        # Tail pre-drain: the auto kernel-tail drain on SP must observe
        # every DMA queue and engine tick; feed SP one dependency per
        # pre-drain (same-proc waits merge) so the auto drain ends at zero.
        tail_deps = [i for i in nc.inst_map.values()
                     if type(i).__name__ == "InstDMACopy"]
        _ctl = {"InstUnconditionalBranch", "InstRegisterMove", "InstDrain",
                "InstEventSemaphore", "InstCall", "InstCompareBranch"}
        snap = [i for i in nc.inst_map.values()
                if type(i).__name__ not in _ctl]
        for eng in ("DVE", "Activation"):
            last_e = [i for i in snap
                      if str(getattr(i, "engine", "")).endswith(eng)]
            if last_e:
                tail_deps.append(last_e[-1])
        tail_deps += [last_mm.ins, scan1.ins]
        for depi in tail_deps:
            dr = nc.sync.drain(fusable=False)
            bass._add_dep_helper(dr.ins, depi, sync=True,
                                 reason="tail pre-drain absorber")
    _assert_wait_budget(nc)
    return nc


# Instruction families exempt from the 1-sync-wait TPB events header (DMA
# descriptors and drains use the queue sync machinery). Everything that runs
# on a TPB engine sequencer (PE/DVE/ACT/Pool alike) is capacity-1.
_MULTI_WAIT_OK = {"InstDrain",
                  "InstEventSemaphore", "InstUnconditionalBranch",
                  "InstRegisterMove", "InstISA", "InstTensorLoad",
                  "InstTensorSave"}


def _assert_wait_budget(nc):
    bad = []
    for name, inst in nc.inst_map.items():
        ty = type(inst).__name__
        if ty in _MULTI_WAIT_OK:
            continue
        w = inst.sync_info.on_wait if inst.sync_info else []
        if len(w) > 1:
            bad.append((name, ty,
                        [f"{x.ant_name}>={x.wait_value}" for x in w]))
    if bad:
        raise RuntimeError(
            f"{len(bad)} instructions exceed the 1-sync-wait TPB limit, "
            f"first few: {bad[:5]}")


def _prep_core_inputs(Input, W0, Ws, bs, whs, core):
    """Host-side staging for one core: shard batch, transpose layer-0 input,
    pre-transpose weights into lhsT layout."""
    bsl = slice(core * BLOC, (core + 1) * BLOC)
    return {
        "xT": np.ascontiguousarray(
            Input[bsl].transpose(0, 2, 1)).astype(np.float16),
        "w0t": np.ascontiguousarray(W0.T).astype(np.float16),
        "wst": np.ascontiguousarray(Ws.transpose(0, 2, 1)).astype(np.float16),
        "bias": np.ascontiguousarray(bs[:, None, :]).astype(np.float16),
        "wbc": np.ascontiguousarray(whs.astype(np.float32)[:, :, None]),
    }


def kernel(Input, W0, Ws, bs, whs):
    include_bias = bool(np.any(bs != 0))
    nc = build(include_bias=include_bias)
    in_maps = [_prep_core_inputs(Input, W0, Ws, bs, whs, r)
               for r in range(NCORES)]
    res = run_bass_kernel_spmd(nc, in_maps, core_ids=list(range(NCORES)))
    parts = [res.results[r]["out"] for r in range(NCORES)]  # [BLOC, H, T] each
    full = np.concatenate(parts, axis=0)  # [B, H, T]
    return np.ascontiguousarray(full.transpose(0, 2, 1)).astype(np.float32)
